# revision 5
# baseline (speedup 1.0000x reference)
"""AssistedExcitation Trainium2 kernel.

out[b,c,h,w] = x[b,c,h,w] + bbox_mask[b,h,w] * mean_c(x[b,:,h,w])

Data-parallel over 8 NeuronCores: 2 images per core, no collectives.
Per core: stream x shard [2,256,16384] through SBUF in [128, CHUNK]
tiles (channels on partitions), channel-sum via matmul with a 1/256
column vector, bbox mask rasterized on device via outer-product
matmuls, mask*mean broadcast back across channels via a K=1 matmul.
"""

import sys

sys.path.insert(0, "/opt/trn_rl_repo")

import numpy as np

import concourse.bacc as bacc
import concourse.bass as bass
import concourse.mybir as mybir
import concourse.tile as tile
from concourse import bass_utils

# Problem constants (hardcoded per harness contract)
B, C, H, W = 16, 256, 128, 128
N_BOX = 320
N_CORES = 8
B_SHARD = B // N_CORES  # 2 images per core
HW = H * W  # 16384
P = 128  # partitions
CHUNK = 2048  # free-dim elements per x tile (16 rows of the image)
N_CHUNK = HW // CHUNK  # 8
SUB = 512  # matmul moving free-dim (one PSUM bank of f32)
N_SUB = CHUNK // SUB  # 4
NBOX_PAD = 384  # 320 boxes padded to 3 tiles of 128
N_BOX_TILES = NBOX_PAD // P  # 3
ALPHA = 1.0

F32 = mybir.dt.float32


def build_nc():
    """Build the per-core Bass graph (SPMD: same graph on all 8 cores)."""
    nc = bacc.Bacc(None, target_bir_lowering=False)

    x = nc.declare_dram_parameter("x", [B_SHARD, C, HW], F32, isOutput=False)
    boxes = nc.declare_dram_parameter("boxes", [NBOX_PAD, 4], F32, isOutput=False)
    sel = nc.declare_dram_parameter("sel", [NBOX_PAD, 2], F32, isOutput=False)
    out = nc.declare_dram_parameter("out", [B_SHARD, C, HW], F32, isOutput=True)

    with tile.TileContext(nc) as tc:
        with (
            tc.tile_pool(name="const", bufs=1) as constp,
            tc.tile_pool(name="boxp", bufs=1) as boxp,
            tc.tile_pool(name="maskp", bufs=1) as maskp,
            tc.tile_pool(name="xp", bufs=3) as xp,
            tc.tile_pool(name="outp", bufs=3) as outp,
            tc.tile_pool(name="smallp", bufs=4) as smallp,
            tc.tile_pool(name="ps_s", bufs=2, space=bass.MemorySpace.PSUM) as ps_s,
            tc.tile_pool(name="ps_b", bufs=2, space=bass.MemorySpace.PSUM) as ps_b,
            tc.tile_pool(name="ps_m", bufs=2, space=bass.MemorySpace.PSUM) as ps_m,
        ):
            # --- constants ---
            wsum = constp.tile([P, 1], F32)  # 1/C column -> channel mean via matmul
            nc.vector.memset(wsum[:], ALPHA / C)
            ones1 = constp.tile([1, P], F32)  # K=1 broadcast row
            nc.vector.memset(ones1[:], 1.0)
            iota_i = constp.tile([P, P], mybir.dt.int32)
            nc.gpsimd.iota(iota_i[:], pattern=[[1, P]], base=0, channel_multiplier=0)
            iota_f = constp.tile([P, P], F32)  # each partition: 0..127 along free
            nc.vector.tensor_copy(iota_f[:], iota_i[:])

            # --- box rasterization setup (tiny) ---
            # Per box n (on partitions): vx1m1 = (xc-bw/2)*W - 1, vx2 = (xc+bw/2)*W
            # cols[n,w] = (w > vx1m1) & (w <= vx2)   (== ref's clamped-int test)
            # valid = (#cols>=2) & (#rows>=2)        (== ref's x2>x1 & y2>y1)
            rows_sel = [[None] * N_BOX_TILES for _ in range(B_SHARD)]
            cols_val = [None] * N_BOX_TILES
            for t in range(N_BOX_TILES):
                bx = boxp.tile([P, 4], F32, tag=f"bx{t}")
                nc.sync.dma_start(bx[:], boxes[t * P : (t + 1) * P, :])
                st = boxp.tile([P, 2], F32, tag=f"st{t}")
                nc.sync.dma_start(st[:], sel[t * P : (t + 1) * P, :])

                xc, yc, bw, bh = (bx[:, i : i + 1] for i in range(4))
                hbw = smallp.tile([P, 1], F32, tag="hbw")
                nc.vector.tensor_scalar_mul(hbw[:], bw, 0.5)
                hbh = smallp.tile([P, 1], F32, tag="hbh")
                nc.vector.tensor_scalar_mul(hbh[:], bh, 0.5)

                def edge(center, half, w_scale, bias, tag):
                    lo = smallp.tile([P, 1], F32, tag=tag + "a")
                    nc.vector.tensor_tensor(
                        lo[:], center, half[:],
                        op=mybir.AluOpType.subtract if bias else mybir.AluOpType.add,
                    )
                    o = smallp.tile([P, 1], F32, tag=tag + "b")
                    if bias:
                        nc.vector.tensor_scalar(
                            o[:], lo[:], float(w_scale), -1.0,
                            op0=mybir.AluOpType.mult, op1=mybir.AluOpType.add,
                        )
                    else:
                        nc.vector.tensor_scalar_mul(o[:], lo[:], float(w_scale))
                    return o

                vx1m1 = edge(xc, hbw, W, True, "vx1")
                vx2 = edge(xc, hbw, W, False, "vx2")
                vy1m1 = edge(yc, hbh, H, True, "vy1")
                vy2 = edge(yc, hbh, H, False, "vy2")

                def member(lo_m1, hi, tag):
                    g1 = smallp.tile([P, P], F32, tag=tag + "g1")
                    nc.vector.tensor_scalar(
                        g1[:], iota_f[:], lo_m1[:], None, op0=mybir.AluOpType.is_gt
                    )
                    g2 = smallp.tile([P, P], F32, tag=tag + "g2")
                    nc.vector.tensor_scalar(
                        g2[:], iota_f[:], hi[:], None, op0=mybir.AluOpType.is_le
                    )
                    m = boxp.tile([P, P], F32, tag=tag + "m")
                    nc.vector.tensor_mul(m[:], g1[:], g2[:])
                    return m

                cols = member(vx1m1, vx2, f"c{t}")
                rows = member(vy1m1, vy2, f"r{t}")

                def count_ok(m, tag):
                    cnt = smallp.tile([P, 1], F32, tag=tag + "cnt")
                    nc.vector.tensor_reduce(
                        cnt[:], m[:], axis=mybir.AxisListType.X, op=mybir.AluOpType.add
                    )
                    ok = smallp.tile([P, 1], F32, tag=tag + "ok")
                    nc.vector.tensor_scalar(
                        ok[:], cnt[:], 1.5, None, op0=mybir.AluOpType.is_ge
                    )
                    return ok

                cok = count_ok(cols, f"c{t}")
                rok = count_ok(rows, f"r{t}")
                vfac = smallp.tile([P, 1], F32, tag="vfac")
                nc.vector.tensor_mul(vfac[:], cok[:], rok[:])

                cv = boxp.tile([P, P], F32, tag=f"cv{t}")
                nc.vector.tensor_scalar(
                    cv[:], cols[:], vfac[:], None, op0=mybir.AluOpType.mult
                )
                cols_val[t] = cv
                for j in range(B_SHARD):
                    rs = boxp.tile([P, P], F32, tag=f"rs{t}_{j}")
                    nc.vector.tensor_scalar(
                        rs[:], rows[:], st[:, j : j + 1], None, op0=mybir.AluOpType.mult
                    )
                    rows_sel[j][t] = rs

            # --- per-image mask: psum[h,w] = sum_n rows[n,h]*cols[n,w]; clamp; ---
            # reshape [128,128] -> [8,2048] so chunk rows align with x tiles.
            masks = []
            for j in range(B_SHARD):
                pm = ps_m.tile([P, W], F32)
                for t in range(N_BOX_TILES):
                    nc.tensor.matmul(
                        pm[:], rows_sel[j][t][:], cols_val[t][:],
                        start=(t == 0), stop=(t == N_BOX_TILES - 1),
                    )
                msb = maskp.tile([P, W], F32, tag=f"msb{j}")
                nc.vector.tensor_scalar_min(msb[:], pm[:], 1.0)
                masks.append(msb)

            # --- main stream: 2 images x 8 chunks of [256, 2048] ---
            rows_per_chunk = CHUNK // W  # 16 image rows per chunk
            for b in range(B_SHARD):
                for ci in range(N_CHUNK):
                    csl = slice(ci * CHUNK, (ci + 1) * CHUNK)
                    A = xp.tile([P, CHUNK], F32, tag="A")
                    nc.sync.dma_start(A[:], x[b, 0:P, csl])
                    Bt = xp.tile([P, CHUNK], F32, tag="B")
                    nc.sync.dma_start(Bt[:], x[b, P:C, csl])
                    # flatten this chunk's 16 mask rows [16,128] -> [1,2048]
                    mf = smallp.tile([1, CHUNK], F32, tag="mf")
                    nc.sync.dma_start(
                        mf[:],
                        masks[b][ci * rows_per_chunk : (ci + 1) * rows_per_chunk, :],
                    )
                    oA = outp.tile([P, CHUNK], F32, tag="oA")
                    oB = outp.tile([P, CHUNK], F32, tag="oB")
                    for s in range(N_SUB):
                        ssl = slice(s * SUB, (s + 1) * SUB)
                        ps = ps_s.tile([1, SUB], F32)
                        nc.tensor.matmul(ps[:], wsum[:], A[:, ssl], start=True, stop=False)
                        nc.tensor.matmul(ps[:], wsum[:], Bt[:, ssl], start=False, stop=True)
                        ad = smallp.tile([1, SUB], F32, tag="ad")
                        nc.vector.tensor_mul(ad[:], ps[0:1, :], mf[0:1, ssl])
                        pb = ps_b.tile([P, SUB], F32)
                        nc.tensor.matmul(pb[:], ones1[:], ad[0:1, :], start=True, stop=True)
                        nc.vector.tensor_add(oA[:, ssl], A[:, ssl], pb[:])
                        nc.vector.tensor_add(oB[:, ssl], Bt[:, ssl], pb[:])
                    nc.sync.dma_start(out[b, 0:P, csl], oA[:])
                    nc.sync.dma_start(out[b, P:C, csl], oB[:])

    return nc


def _host_prep(x, bboxes, batch_idx):
    """Shard inputs; build padded box/selector arrays (tiny host-side prep)."""
    x = np.ascontiguousarray(np.asarray(x, dtype=np.float32)).reshape(B, C, HW)
    bboxes = np.asarray(bboxes, dtype=np.float32)
    batch_idx = np.asarray(batch_idx).astype(np.int64)

    boxes_pad = np.zeros((NBOX_PAD, 4), dtype=np.float32)
    boxes_pad[:N_BOX] = bboxes

    in_maps = []
    for i in range(N_CORES):
        sel_i = np.zeros((NBOX_PAD, 2), dtype=np.float32)
        for j in range(B_SHARD):
            sel_i[:N_BOX, j] = (batch_idx == (i * B_SHARD + j)).astype(np.float32)
        in_maps.append(
            {
                "x": np.ascontiguousarray(x[i * B_SHARD : (i + 1) * B_SHARD]),
                "boxes": boxes_pad,
                "sel": sel_i,
            }
        )
    return in_maps


def kernel(x, bboxes, batch_idx):
    in_maps = _host_prep(x, bboxes, batch_idx)
    nc = build_nc()
    nc.finalize()
    res = bass_utils.run_bass_kernel_spmd(nc, in_maps, core_ids=list(range(N_CORES)))
    shards = [res.results[i]["out"] for i in range(N_CORES)]
    return np.concatenate(shards, axis=0).reshape(B, C, H, W)


if __name__ == "__main__":
    nc = build_nc()
    nc.finalize()
    print("built ok:", len(nc.inst_map), "instructions")


# revision 10
# speedup vs baseline: 197.8859x; 197.8859x over previous
"""AssistedExcitation Trainium2 kernel.

out[b,c,h,w] = x[b,c,h,w] + bbox_mask[b,h,w] * mean_c(x[b,:,h,w])

Data-parallel over 8 NeuronCores: 2 images per core, no collectives.
Per core: stream x shard [2,256,16384] through SBUF in [128, CHUNK]
tiles (channels on partitions), channel-sum via matmul with a 1/256
column vector, bbox mask rasterized on device via outer-product
matmuls, mask*mean broadcast back across channels via a K=1 matmul.
"""

import sys

sys.path.insert(0, "/opt/trn_rl_repo")

import numpy as np

import concourse.bacc as bacc
import concourse.bass as bass
import concourse.mybir as mybir
import concourse.tile as tile
from concourse import bass_utils

# Problem constants (hardcoded per harness contract)
B, C, H, W = 16, 256, 128, 128
N_BOX = 320
N_CORES = 8
B_SHARD = B // N_CORES  # 2 images per core
HW = H * W  # 16384
P = 128  # partitions
CHUNK = 2048  # free-dim elements per x tile (16 rows of the image)
N_CHUNK = HW // CHUNK  # 8
SUB = 512  # matmul moving free-dim (one PSUM bank of f32)
N_SUB = CHUNK // SUB  # 4
NBOX_PAD = 384  # 320 boxes padded to 3 tiles of 128
N_BOX_TILES = NBOX_PAD // P  # 3
ALPHA = 1.0

F32 = mybir.dt.float32
F32R = mybir.dt.float32r  # relaxed-precision fp32 matmul: 4x PE throughput
BF16 = mybir.dt.bfloat16


def _r(ap):
    return ap.bitcast(F32R)


def build_nc():
    """Build the per-core Bass graph (SPMD: same graph on all 8 cores)."""
    nc = bacc.Bacc(None, target_bir_lowering=False)

    x = nc.declare_dram_parameter("x", [B_SHARD, C, HW], F32R, isOutput=False)
    boxes = nc.declare_dram_parameter("boxes", [NBOX_PAD, 4], F32, isOutput=False)
    sel = nc.declare_dram_parameter("sel", [NBOX_PAD, 2], F32, isOutput=False)
    wsum_d = nc.declare_dram_parameter("wsum", [P, 1], F32R, isOutput=False)
    out = nc.declare_dram_parameter("out", [B_SHARD, C, HW], F32, isOutput=True)

    with tile.TileContext(nc) as tc:
        with (
            tc.tile_pool(name="const", bufs=1) as constp,
            tc.tile_pool(name="boxp", bufs=1) as boxp,
            tc.tile_pool(name="maskp", bufs=1) as maskp,
            tc.tile_pool(name="xp", bufs=3) as xp,
            tc.tile_pool(name="outp", bufs=3) as outp,
            tc.tile_pool(name="smallp", bufs=4) as smallp,
            tc.tile_pool(name="ps_s", bufs=2, space=bass.MemorySpace.PSUM) as ps_s,
            tc.tile_pool(name="ps_b", bufs=2, space=bass.MemorySpace.PSUM) as ps_b,
            tc.tile_pool(name="ps_m", bufs=2, space=bass.MemorySpace.PSUM) as ps_m,
        ):
            # --- constants ---
            wsum = constp.tile([P, 1], F32R)  # 1/C column -> channel mean
            nc.sync.dma_start(wsum[:], wsum_d[:])
            ones1 = constp.tile([1, P], BF16)  # K=1 broadcast row
            nc.vector.memset(ones1[:], 1.0)
            iota_i = constp.tile([P, P], mybir.dt.int32)
            nc.gpsimd.iota(iota_i[:], pattern=[[1, P]], base=0, channel_multiplier=0)
            iota_f = constp.tile([P, P], F32)  # each partition: 0..127 along free
            nc.vector.tensor_copy(iota_f[:], iota_i[:])

            # --- box rasterization setup (tiny) ---
            # Per box n (on partitions): vx1m1 = (xc-bw/2)*W - 1, vx2 = (xc+bw/2)*W
            # cols[n,w] = (w > vx1m1) & (w <= vx2)   (== ref's clamped-int test)
            # valid = (#cols>=2) & (#rows>=2)        (== ref's x2>x1 & y2>y1)
            rows_sel = [[None] * N_BOX_TILES for _ in range(B_SHARD)]
            cols_val = [None] * N_BOX_TILES
            for t in range(N_BOX_TILES):
                bx = boxp.tile([P, 4], F32, tag=f"bx{t}")
                nc.sync.dma_start(bx[:], boxes[t * P : (t + 1) * P, :])
                st = boxp.tile([P, 2], F32, tag=f"st{t}")
                nc.sync.dma_start(st[:], sel[t * P : (t + 1) * P, :])

                xc, yc, bw, bh = (bx[:, i : i + 1] for i in range(4))
                hbw = smallp.tile([P, 1], F32, tag="hbw")
                nc.vector.tensor_scalar_mul(hbw[:], bw, 0.5)
                hbh = smallp.tile([P, 1], F32, tag="hbh")
                nc.vector.tensor_scalar_mul(hbh[:], bh, 0.5)

                def edge(center, half, w_scale, bias, tag):
                    lo = smallp.tile([P, 1], F32, tag=tag + "a")
                    nc.vector.tensor_tensor(
                        lo[:], center, half[:],
                        op=mybir.AluOpType.subtract if bias else mybir.AluOpType.add,
                    )
                    o = smallp.tile([P, 1], F32, tag=tag + "b")
                    if bias:
                        nc.vector.tensor_scalar(
                            o[:], lo[:], float(w_scale), -1.0,
                            op0=mybir.AluOpType.mult, op1=mybir.AluOpType.add,
                        )
                    else:
                        nc.vector.tensor_scalar_mul(o[:], lo[:], float(w_scale))
                    return o

                vx1m1 = edge(xc, hbw, W, True, "vx1")
                vx2 = edge(xc, hbw, W, False, "vx2")
                vy1m1 = edge(yc, hbh, H, True, "vy1")
                vy2 = edge(yc, hbh, H, False, "vy2")

                def member(lo_m1, hi, tag):
                    g1 = smallp.tile([P, P], F32, tag=tag + "g1")
                    nc.vector.tensor_scalar(
                        g1[:], iota_f[:], lo_m1[:], None, op0=mybir.AluOpType.is_gt
                    )
                    g2 = smallp.tile([P, P], F32, tag=tag + "g2")
                    nc.vector.tensor_scalar(
                        g2[:], iota_f[:], hi[:], None, op0=mybir.AluOpType.is_le
                    )
                    m = boxp.tile([P, P], F32, tag=tag + "m")
                    nc.vector.tensor_mul(m[:], g1[:], g2[:])
                    return m

                cols = member(vx1m1, vx2, f"c{t}")
                rows = member(vy1m1, vy2, f"r{t}")

                def count_ok(m, tag):
                    cnt = smallp.tile([P, 1], F32, tag=tag + "cnt")
                    nc.vector.tensor_reduce(
                        cnt[:], m[:], axis=mybir.AxisListType.X, op=mybir.AluOpType.add
                    )
                    ok = smallp.tile([P, 1], F32, tag=tag + "ok")
                    nc.vector.tensor_scalar(
                        ok[:], cnt[:], 1.5, None, op0=mybir.AluOpType.is_ge
                    )
                    return ok

                cok = count_ok(cols, f"c{t}")
                rok = count_ok(rows, f"r{t}")
                vfac = smallp.tile([P, 1], F32, tag="vfac")
                nc.vector.tensor_mul(vfac[:], cok[:], rok[:])

                cv = boxp.tile([P, P], F32, tag=f"cv{t}")
                nc.vector.tensor_scalar(
                    cv[:], cols[:], vfac[:], None, op0=mybir.AluOpType.mult
                )
                cols_val[t] = cv
                for j in range(B_SHARD):
                    rs = boxp.tile([P, P], F32, tag=f"rs{t}_{j}")
                    nc.vector.tensor_scalar(
                        rs[:], rows[:], st[:, j : j + 1], None, op0=mybir.AluOpType.mult
                    )
                    rows_sel[j][t] = rs

            # --- per-image mask: psum[h,w] = sum_n rows[n,h]*cols[n,w]; clamp; ---
            # reshape [128,128] -> [8,2048] so chunk rows align with x tiles.
            masks = []
            for j in range(B_SHARD):
                pm = ps_m.tile([P, W], F32)
                for t in range(N_BOX_TILES):
                    nc.tensor.matmul(
                        pm[:], rows_sel[j][t][:], cols_val[t][:],
                        start=(t == 0), stop=(t == N_BOX_TILES - 1),
                    )
                msb = maskp.tile([P, W], F32, tag=f"msb{j}")
                nc.vector.tensor_scalar_min(msb[:], pm[:], 1.0)
                masks.append(msb)

            # --- main stream: 2 images x 8 chunks of [256, 2048] ---
            rows_per_chunk = CHUNK // W  # 16 image rows per chunk
            for b in range(B_SHARD):
                for ci in range(N_CHUNK):
                    csl = slice(ci * CHUNK, (ci + 1) * CHUNK)
                    A = xp.tile([P, CHUNK], F32R, tag="A")
                    nc.sync.dma_start(A[:], x[b, 0:P, csl])
                    Bt = xp.tile([P, CHUNK], F32R, tag="B")
                    nc.sync.dma_start(Bt[:], x[b, P:C, csl])
                    # flatten this chunk's 16 mask rows [16,128] -> [1,2048]
                    mf = smallp.tile([1, CHUNK], F32, tag="mf")
                    nc.tensor.dma_start(
                        mf[:],
                        masks[b][ci * rows_per_chunk : (ci + 1) * rows_per_chunk, :],
                    )
                    oA = outp.tile([P, CHUNK], F32, tag="oA")
                    oB = outp.tile([P, CHUNK], F32, tag="oB")
                    for h in range(CHUNK // (2 * SUB)):  # 1024-wide halves
                        hsl = slice(h * 2 * SUB, (h + 1) * 2 * SUB)
                        ps = ps_s.tile([1, 2 * SUB], F32)
                        for s2 in range(2):
                            ssl = slice((2 * h + s2) * SUB, (2 * h + s2 + 1) * SUB)
                            psl = slice(s2 * SUB, (s2 + 1) * SUB)
                            nc.tensor.matmul(
                                ps[0:1, psl], wsum[:], A[:, ssl], start=True, stop=False
                            )
                            nc.tensor.matmul(
                                ps[0:1, psl], wsum[:], Bt[:, ssl], start=False, stop=True
                            )
                        ad = smallp.tile([1, 2 * SUB], BF16, tag="ad")
                        nc.vector.tensor_mul(ad[:], ps[0:1, :], mf[0:1, hsl])
                        for s2 in range(2):
                            ssl = slice((2 * h + s2) * SUB, (2 * h + s2 + 1) * SUB)
                            psl = slice(s2 * SUB, (s2 + 1) * SUB)
                            pb = ps_b.tile([P, SUB], F32)
                            nc.tensor.matmul(
                                pb[:], ones1[:], ad[0:1, psl], start=True, stop=True
                            )
                            nc.vector.tensor_add(
                                oA[:, ssl], A[:, ssl].bitcast(F32), pb[:]
                            )
                            nc.vector.tensor_add(
                                oB[:, ssl], Bt[:, ssl].bitcast(F32), pb[:]
                            )
                    nc.scalar.dma_start(out[b, 0:P, csl], oA[:])
                    nc.scalar.dma_start(out[b, P:C, csl], oB[:])

    return nc


def _host_prep(x, bboxes, batch_idx):
    """Shard inputs; build padded box/selector arrays (tiny host-side prep)."""
    x = np.ascontiguousarray(np.asarray(x, dtype=np.float32)).reshape(B, C, HW)
    bboxes = np.asarray(bboxes, dtype=np.float32)
    batch_idx = np.asarray(batch_idx).astype(np.int64)

    boxes_pad = np.zeros((NBOX_PAD, 4), dtype=np.float32)
    boxes_pad[:N_BOX] = bboxes

    in_maps = []
    for i in range(N_CORES):
        sel_i = np.zeros((NBOX_PAD, 2), dtype=np.float32)
        for j in range(B_SHARD):
            sel_i[:N_BOX, j] = (batch_idx == (i * B_SHARD + j)).astype(np.float32)
        in_maps.append(
            {
                "x": np.ascontiguousarray(x[i * B_SHARD : (i + 1) * B_SHARD]),
                "boxes": boxes_pad,
                "sel": sel_i,
                "wsum": np.full((P, 1), ALPHA / C, dtype=np.float32),
            }
        )
    return in_maps


def kernel(x, bboxes, batch_idx):
    in_maps = _host_prep(x, bboxes, batch_idx)
    nc = build_nc()
    nc.finalize()
    res = bass_utils.run_bass_kernel_spmd(nc, in_maps, core_ids=list(range(N_CORES)))
    shards = [res.results[i]["out"] for i in range(N_CORES)]
    return np.concatenate(shards, axis=0).reshape(B, C, H, W)


if __name__ == "__main__":
    nc = build_nc()
    nc.finalize()
    print("built ok:", len(nc.inst_map), "instructions")
